# revision 3
# baseline (speedup 1.0000x reference)
"""MultiBoxLoss (SSD) Bass/Trainium2 kernel, v2.

Data-parallel over batch: 64 rows -> 8 cores x 8 rows. Each core computes
partial sums [loss_l_num, sum_pos_lse, sum_pos_conf_gt, S_neg_total, num_pos]
fully on device; host combines 8x5 scalars and divides by N.

v2 redesign vs baseline:
- argmax-over-truth replaced by a max-mask (is_ge vs row max) + payload
  gathers via bf16 mult + max-reduce (one-hot => max == select).
- forced-prior override folded into iou as +2*onehot(best prior per truth),
  with a small [16x16] dedupe (last truth wins) instead of full-grid ovr.
- best-prior-per-truth via global-max mask + min-linear-index reduction.
- elementwise work split across DVE / Pool(gpsimd) / Act engines.
- conf-gt one-hot on Pool, exp on Act (bf16 out), serow reduce bf16 on DVE.
- Ln deferred to a phase B so the Act engine needs only 2 table loads.
- mining bisection on bf16 loss values, 16 iterations.
"""

from contextlib import ExitStack

import numpy as np

import concourse.bass as bass
import concourse.bacc as bacc
import concourse.tile as tile
from concourse import mybir
from concourse import bass_utils

F32 = mybir.dt.float32
BF16 = mybir.dt.bfloat16
I32 = mybir.dt.int32
U8 = mybir.dt.uint8
OP = mybir.AluOpType
AF = mybir.ActivationFunctionType
AX = mybir.AxisListType

B, P, T, C = 64, 25000, 16, 81
NCORES = 8
R = B // NCORES          # rows per core
NP = 196                 # priors per partition
PADP = 128 * NP          # 25088
FULLP = P // NP          # 127 full partitions
TAILF = P - FULLP * NP   # 108 valid f on partition 127
GC = 28                  # conf chunk f-width (7 chunks of 28 = 196)
NCH = NP // GC           # 7
MCH = PADP // 16         # 1568 mining width; 8 rows * 16 chunks = 128
N_ITERS = 14
BIGC = 16777216.0        # 2^24, exact in f32


def build_program():
    nc = bacc.Bacc("TRN2", target_bir_lowering=False, debug=False)
    loc = nc.dram_tensor("loc", [R, P, 4], F32, kind="ExternalInput").ap()
    conf = nc.dram_tensor("conf", [R, P, C], F32, kind="ExternalInput").ap()
    priors = nc.dram_tensor("priors", [P, 4], F32, kind="ExternalInput").ap()
    targets = nc.dram_tensor("targets", [R, T, 5], F32, kind="ExternalInput").ap()
    out = nc.dram_tensor("out", [1, 8], F32, kind="ExternalOutput").ap()
    scratch = nc.dram_tensor("scratch", [R, PADP], BF16, kind="Internal").ap()
    smask = nc.dram_tensor("smask", [R, 16, 128, NP], BF16, kind="Internal").ap()
    smct = nc.dram_tensor("smct", [R, 32, PADP], F32, kind="Internal").ap()
    sb16 = nc.dram_tensor("sb16", [R, 16], F32, kind="Internal").ap()
    sg16 = nc.dram_tensor("sg16", [R, 16], F32, kind="Internal").ap()
    sk8 = nc.dram_tensor("sk8", [1, 8], F32, kind="Internal").ap()

    with TileKernel(nc) as tk:
        tk.sb16 = sb16
        tk.sg16 = sg16
        tk.sk8 = sk8
        tk.smask = smask
        tk.smct = smct
        with nc.allow_low_precision("bf16 selections/sums stay within tolerance"):
            tk.build(loc, conf, priors, targets, out, scratch)
    nc.compile()
    return nc


class TileKernel:
    def __init__(self, nc):
        self.nc = nc
        self.ctx = ExitStack()
        self.tc = None

    def __enter__(self):
        self.tc = self.ctx.enter_context(tile.TileContext(self.nc))
        return self

    def __exit__(self, *a):
        return self.ctx.__exit__(*a)

    def build(self, loc, conf, priors, targets, out, scratch):
        nc = self.nc
        tc = self.tc
        ctx = self.ctx
        consts = ctx.enter_context(tc.tile_pool(name="consts", bufs=1))
        res = ctx.enter_context(tc.tile_pool(name="res", bufs=1))
        acc = ctx.enter_context(tc.tile_pool(name="acc", bufs=1))
        psum = ctx.enter_context(tc.tile_pool(name="psum", bufs=4, space="PSUM"))
        self.psum_mc = ctx.enter_context(
            tc.tile_pool(name="psmc", bufs=2, space="PSUM"))
        self.consts = consts
        self.res = res
        self.acc = acc
        self.psum = psum

        # ---------------- constants ----------------
        initpool = consts
        iop_i = initpool.tile([128, 1], I32, tag="c0")
        nc.gpsimd.iota(iop_i, pattern=[[0, 1]], base=0, channel_multiplier=1)
        iop_f = initpool.tile([128, 1], F32, tag="c0f")
        nc.vector.tensor_copy(iop_f, iop_i)
        iom128_i = initpool.tile([128, 128], I32, tag="c1")
        nc.gpsimd.iota(iom128_i, pattern=[[1, 128]], base=0, channel_multiplier=0)
        iomf = initpool.tile([128, 128], F32, tag="c1f")
        nc.vector.tensor_copy(iomf, iom128_i)
        ident = consts.tile([128, 128], F32, tag="c2")
        nc.vector.tensor_scalar(ident, iomf, iop_f[:, 0:1], None, OP.is_equal)
        lin_i = initpool.tile([128, NP], I32, tag="c3")
        nc.gpsimd.iota(lin_i, pattern=[[1, NP]], base=0, channel_multiplier=NP)
        linf = consts.tile([128, NP], F32, tag="c3f")
        nc.vector.tensor_copy(linf, lin_i)
        linmB = consts.tile([128, NP], F32, tag="c4")
        nc.vector.tensor_scalar(linmB, linf, -BIGC, None, OP.add)
        validm = consts.tile([128, NP], F32, tag="c5")
        nc.vector.tensor_scalar(validm, linf, float(P), None, OP.is_lt)
        io81_i = initpool.tile([128, C], I32, tag="c6")
        nc.gpsimd.iota(io81_i, pattern=[[1, C]], base=0, channel_multiplier=0)
        io81b = consts.tile([128, C], BF16, tag="c6b")
        nc.vector.tensor_copy(io81b, io81_i)
        io16_i = initpool.tile([128, 16], I32, tag="c7")
        nc.gpsimd.iota(io16_i, pattern=[[1, 16]], base=0, channel_multiplier=0)
        io16f = initpool.tile([128, 16], F32, tag="c7f")
        nc.vector.tensor_copy(io16f, io16_i)
        io16b = consts.tile([128, 16], BF16, tag="c7b")
        nc.vector.tensor_copy(io16b, io16f)
        tp1b = consts.tile([128, 16], BF16, tag="c7p")
        nc.vector.tensor_scalar(tp1b, io16b, 1.0, None, OP.add)
        upper = consts.tile([128, 16, 16], BF16, tag="c8")
        nc.vector.tensor_tensor(
            upper,
            io16f[:, :, None].to_broadcast([128, 16, 16]),
            io16f[:, None, :].to_broadcast([128, 16, 16]),
            OP.is_lt)

        # mining selectors
        gp_i = initpool.tile([128, 1], I32, tag="c9")
        nc.vector.tensor_scalar(gp_i, iop_i, 4, None, OP.arith_shift_right)
        gp_f = consts.tile([128, 1], F32, tag="c9f")
        nc.vector.tensor_copy(gp_f, gp_i)
        m16_i = initpool.tile([128, 128], I32, tag="c10")
        nc.vector.tensor_scalar(m16_i, iom128_i, 4, None, OP.arith_shift_right)
        m16_f = initpool.tile([128, 128], F32, tag="c10f")
        nc.vector.tensor_copy(m16_f, m16_i)
        G128 = consts.tile([128, 128], F32, tag="c11")
        nc.vector.tensor_scalar(G128, m16_f, gp_f[:, 0:1], None, OP.is_equal)
        iom8_i = initpool.tile([128, 8], I32, tag="c12")
        nc.gpsimd.iota(iom8_i, pattern=[[1, 8]], base=0, channel_multiplier=0)
        iom8_f = initpool.tile([128, 8], F32, tag="c12f")
        nc.vector.tensor_copy(iom8_f, iom8_i)
        sel8 = consts.tile([128, 8], F32, tag="c13")
        nc.vector.tensor_scalar(sel8, iom8_f, gp_f[:, 0:1], None, OP.is_equal)
        and_i = initpool.tile([128, 1], I32, tag="c14")
        nc.vector.tensor_scalar(and_i, iop_i, 15, None, OP.bitwise_and)
        selone = consts.tile([128, 1], F32, tag="c14f")
        nc.vector.tensor_copy(selone, and_i)
        nc.vector.tensor_scalar(selone, selone, 0.0, None, OP.is_equal)
        ones_128x1 = consts.tile([128, 1], F32, tag="c15")
        nc.vector.memset(ones_128x1, 1.0)

        # ---------------- priors (shared across rows) ----------------
        pri = initpool.tile([128, NP, 4], F32, tag="pri")
        nc.vector.memset(pri[96:128, TAILF:NP, :], 1e-3)
        nc.vector.memset(pri[96:128, TAILF:NP, 0:2], -100.0)
        nc.sync.dma_start(
            out=pri[:FULLP, :, :],
            in_=priors[: FULLP * NP, :].rearrange("(p f) c -> p f c", f=NP),
        )
        nc.sync.dma_start(
            out=pri[FULLP : FULLP + 1, :TAILF, :],
            in_=priors[FULLP * NP : P, :].rearrange("(p f) c -> p f c", p=1),
        )
        pcx, pcy, pw, ph = (pri[:, :, i] for i in range(4))
        px0 = consts.tile([128, NP], F32, tag="px0")
        nc.vector.scalar_tensor_tensor(px0, pw, -0.5, pcx, OP.mult, OP.add)
        py0 = consts.tile([128, NP], F32, tag="py0")
        nc.vector.scalar_tensor_tensor(py0, ph, -0.5, pcy, OP.mult, OP.add)
        px1 = consts.tile([128, NP], F32, tag="px1")
        nc.vector.scalar_tensor_tensor(px1, pw, 0.5, pcx, OP.mult, OP.add)
        py1 = consts.tile([128, NP], F32, tag="py1")
        nc.vector.scalar_tensor_tensor(py1, ph, 0.5, pcy, OP.mult, OP.add)
        parea = consts.tile([128, NP], F32, tag="parea")
        nc.vector.tensor_tensor(parea, pw, ph, OP.mult)
        rw10 = consts.tile([128, NP], F32, tag="rw10")
        nc.vector.reciprocal(rw10, pw)
        nc.vector.tensor_scalar(rw10, rw10, 10.0, None, OP.mult)
        rh10 = consts.tile([128, NP], F32, tag="rh10")
        nc.vector.reciprocal(rh10, ph)
        nc.vector.tensor_scalar(rh10, rh10, 10.0, None, OP.mult)
        lnpw5 = consts.tile([128, NP], F32, tag="lnpw5")
        nc.scalar.activation(lnpw5, pw, AF.Ln)
        nc.vector.tensor_scalar(lnpw5, lnpw5, 5.0, None, OP.mult)
        lnph5 = consts.tile([128, NP], F32, tag="lnph5")
        nc.scalar.activation(lnph5, ph, AF.Ln)
        nc.vector.tensor_scalar(lnph5, lnph5, 5.0, None, OP.mult)
        px0b = consts.tile([128, NP], BF16, tag="px0b")
        nc.scalar.copy(px0b, px0)
        py0b = consts.tile([128, NP], BF16, tag="py0b")
        nc.scalar.copy(py0b, py0)
        px1b = consts.tile([128, NP], BF16, tag="px1b")
        nc.scalar.copy(px1b, px1)
        py1b = consts.tile([128, NP], BF16, tag="py1b")
        nc.scalar.copy(py1b, py1)
        pareab = consts.tile([128, NP], BF16, tag="pareab")
        nc.scalar.copy(pareab, parea)
        self.cst = dict(io16b=io16b, tp1b=tp1b,
                        px0=px0b, py0=py0b, px1=px1b, py1=py1b, parea=pareab,
                        rw10=rw10, rh10=rh10, lnpw5=lnpw5, lnph5=lnph5,
                        linf=linf, linmB=linmB, validm=validm, io81b=io81b,
                        upper=upper, ident=ident, pcx=pcx, pcy=pcy,
                        G128=G128, sel8=sel8, selone=selone,
                        ones_128x1=ones_128x1)

        work = ctx.enter_context(tc.tile_pool(name="work", bufs=1))
        small = ctx.enter_context(tc.tile_pool(name="small", bufs=2))
        workd = ctx.enter_context(tc.tile_pool(name="workd", bufs=2))
        self.work = work
        self.small = small
        self.workd = workd

        # conf chunk double buffers (memset pad partition once per buffer)
        self.cf = [consts.tile([128, GC, C], F32, tag=f"cf{i}", name=f"cf{i}") for i in range(2)]
        self.et = [consts.tile([128, GC, C], BF16, tag=f"et{i}", name=f"et{i}") for i in range(2)]
        self.mk = [consts.tile([128, GC, C], BF16, tag=f"mk{i}", name=f"mk{i}") for i in range(2)]
        self.cfb = [consts.tile([128, GC, C], BF16, tag=f"cfb{i}", name=f"cfb{i}") for i in range(2)]
        for i in range(2):
            nc.gpsimd.memset(self.cf[i][96:128, :, :], 0.0)

        # ---------------- accumulators ----------------
        llacc = acc.tile([128, 1], F32, tag="llacc")
        nc.vector.memset(llacc, 0.0)
        blacc = acc.tile([128, 1], F32, tag="blacc")
        nc.vector.memset(blacc, 0.0)
        bcacc = acc.tile([128, 1], F32, tag="bcacc")
        nc.vector.memset(bcacc, 0.0)
        npmat = acc.tile([128, 8], F32, tag="npmat")
        self.llacc, self.blacc, self.bcacc, self.npmat = llacc, blacc, bcacc, npmat

        # resident per-row tiles
        self.serow = [res.tile([128, NP], BF16, tag=f"se{r}", name=f"se{r}") for r in range(R)]
        self.c0row = [res.tile([128, NP], BF16, tag=f"c0{r}", name=f"c0{r}") for r in range(R)]
        self.posr = [res.tile([128, NP], F32, tag=f"po{r}", name=f"po{r}") for r in range(R)]
        # gathered payloads [ux, vx, uy, vy, lab, 0] per row, ch-major layout
        self.mcres = [res.tile([128, 5, NP], BF16, tag=f"mc{r}", name=f"mc{r}")
                      for r in range(R)]

        # ---------------- phase A: per-row match + conf ----------------
        for r in range(R):
            self.rowA(r, loc, conf, targets)

        # ---------------- phase B: deferred Ln work ----------------
        for r in range(R):
            self.rowB(r, loc, scratch)

        # ---------------- phase C: mining + output ----------------
        self.mining(scratch, out)

    # ------------------------------------------------------------------
    def rowA(self, r, loc, conf, targets):
        nc = self.nc
        cst = self.cst
        work = self.work
        small = self.small
        psum = self.psum
        b3 = lambda ap: ap[:, :, None].to_broadcast([128, NP, 16])
        t3 = lambda ap: ap[:, None, :].to_broadcast([128, NP, 16])

        # --- truths broadcast to all partitions via stride-0 DMA ---
        tgt = small.tile([128, T, 5], F32, tag="tgt")
        tr = targets[r]
        nc.sync.dma_start(
            out=tgt,
            in_=bass.AP(tensor=tr.tensor, offset=tr.offset,
                        ap=[[0, 128]] + list(tr.ap)))
        tx0, ty0, tx1, ty1, tlab = (tgt[:, :, i] for i in range(5))
        tx0b = small.tile([128, 16], BF16, tag="tx0b")
        nc.vector.tensor_copy(tx0b, tx0)
        ty0b = small.tile([128, 16], BF16, tag="ty0b")
        nc.vector.tensor_copy(ty0b, ty0)
        tx1b = small.tile([128, 16], BF16, tag="tx1b")
        nc.vector.tensor_copy(tx1b, tx1)
        ty1b = small.tile([128, 16], BF16, tag="ty1b")
        nc.vector.tensor_copy(ty1b, ty1)
        twx = small.tile([128, 16], BF16, tag="twx")
        nc.vector.tensor_tensor(twx, tx1b, tx0b, OP.subtract)
        thy = small.tile([128, 16], BF16, tag="thy")
        nc.vector.tensor_tensor(thy, ty1b, ty0b, OP.subtract)
        tarea = small.tile([128, 16], BF16, tag="tarea")
        nc.vector.tensor_tensor(tarea, twx, thy, OP.mult)
        uxb = small.tile([128, 16], BF16, tag="uxb")
        nc.vector.tensor_tensor(uxb, tx0b, tx1b, OP.add)
        uyb = small.tile([128, 16], BF16, tag="uyb")
        nc.vector.tensor_tensor(uyb, ty0b, ty1b, OP.add)
        labb = small.tile([128, 16], BF16, tag="labb")
        nc.vector.tensor_copy(labb, tlab)

        # --- IoU [128, NP, 16] ---
        bx1 = work.tile([128, NP, 16], BF16, tag="bx1")
        bx0 = work.tile([128, NP, 16], BF16, tag="bx0")
        by1 = work.tile([128, NP, 16], BF16, tag="by1")
        by0 = work.tile([128, NP, 16], BF16, tag="by0")
        binter = self.workd.tile([128, NP, 16], BF16, tag="binter")
        nc.vector.tensor_tensor(bx1, b3(cst["px1"]), t3(tx1b), OP.min)
        nc.vector.tensor_tensor(bx0, b3(cst["px0"]), t3(tx0b), OP.max)
        nc.vector.tensor_tensor(bx1, bx1, bx0, OP.subtract)       # wx
        nc.scalar.activation(bx1, bx1, AF.Relu)
        nc.vector.tensor_tensor(by1, b3(cst["py1"]), t3(ty1b), OP.min)
        nc.vector.tensor_tensor(by0, b3(cst["py0"]), t3(ty0b), OP.max)
        nc.gpsimd.tensor_tensor(by1, by1, by0, OP.subtract)       # wy
        nc.vector.tensor_tensor(binter, bx1, by1, OP.mult)
        nc.scalar.activation(binter, binter, AF.Relu)             # inter
        bu = by0  # my0 dead after wy; reuse its storage for union
        nc.gpsimd.tensor_tensor(bu, b3(cst["parea"]), t3(tarea), OP.add)
        nc.vector.tensor_tensor(bu, bu, binter, OP.subtract)      # union
        nc.vector.reciprocal(bu, bu)
        nc.vector.tensor_tensor(binter, binter, bu, OP.mult)      # iou

        # --- best prior per truth: global max then min linear index ---
        pf1w = work.tile([128, NP, 16], BF16, tag="bprod")
        pf1 = pf1w[:, 0:98, :]
        nc.vector.tensor_tensor(pf1, binter[:, 0:98, :], binter[:, 98:196, :],
                                OP.max)
        pf2 = work.tile([128, 49, 16], BF16, tag="pf2")
        nc.vector.tensor_tensor(pf2, pf1[:, 0:49, :], pf1[:, 49:98, :], OP.max)
        nc.vector.tensor_tensor(pf1[:, 0:24, :], pf2[:, 0:24, :],
                                pf2[:, 25:49, :], OP.max)
        nc.vector.tensor_tensor(pf1[:, 0:24, :], pf1[:, 0:24, :],
                                pf2[:, 24:25, :].to_broadcast([128, 24, 16]),
                                OP.max)
        pmax = small.tile([128, 16], F32, tag="pmax")
        nc.vector.tensor_reduce(
            pmax, pf1[:, 0:24, :].rearrange("p f t -> p t f"), AX.X, OP.max)
        tp_ps = psum.tile([16, 128], F32, tag="ps")
        nc.tensor.transpose(tp_ps, pmax, cst["ident"])
        gmaxs = small.tile([16, 1], F32, tag="gmaxs")
        nc.vector.tensor_reduce(gmaxs, tp_ps, AX.X, OP.max)
        sgr = self.sg16[r]
        nc.sync.dma_start(out=sgr.rearrange("t -> t ()"), in_=gmaxs)
        gmaxB = small.tile([128, 16], F32, tag="gmaxB")
        nc.sync.dma_start(
            out=gmaxB,
            in_=bass.AP(tensor=sgr.tensor, offset=sgr.offset,
                        ap=[[0, 128]] + list(sgr.ap)))
        gmaxBb = small.tile([128, 16], BF16, tag="gmaxBb")
        nc.vector.tensor_copy(gmaxBb, gmaxB)
        nc.vector.tensor_tensor(bx0, binter, t3(gmaxBb), OP.is_ge)   # mask_g
        bglf = work.tile([128, NP, 16], F32, tag="bglf")
        nc.gpsimd.tensor_tensor(bglf, bx0, b3(cst["linmB"]), OP.mult)  # mgl
        nc.vector.tensor_tensor(bglf[:, 0:98, :], bglf[:, 0:98, :],
                                bglf[:, 98:196, :], OP.min)
        lmin = small.tile([128, 16], F32, tag="lmin")
        nc.vector.tensor_reduce(
            lmin, bglf[:, 0:98, :].rearrange("p f t -> p t f"), AX.X, OP.min)
        tp2_ps = psum.tile([16, 128], F32, tag="ps")
        nc.tensor.transpose(tp2_ps, lmin, cst["ident"])
        bpi0 = small.tile([16, 1], F32, tag="bpi0")
        nc.vector.tensor_reduce(bpi0, tp2_ps, AX.X, OP.min)
        nc.vector.tensor_scalar(bpi0, bpi0, BIGC, None, OP.add)
        sbr = self.sb16[r]
        nc.sync.dma_start(out=sbr.rearrange("t -> t ()"), in_=bpi0)
        bpiB = small.tile([128, 16], F32, tag="bpiB")
        nc.sync.dma_start(
            out=bpiB,
            in_=bass.AP(tensor=sbr.tensor, offset=sbr.offset,
                        ap=[[0, 128]] + list(sbr.ap)))
        # dedupe: drop truth j if a later truth j' maps to the same prior
        eqm = small.tile([128, 16, 16], BF16, tag="eqm")
        nc.vector.tensor_tensor(
            eqm, bpiB[:, :, None].to_broadcast([128, 16, 16]),
            bpiB[:, None, :].to_broadcast([128, 16, 16]), OP.is_equal)
        nc.vector.tensor_tensor(eqm, eqm, cst["upper"], OP.mult)
        bad = small.tile([128, 16], F32, tag="bad")
        nc.vector.tensor_reduce(bad, eqm, AX.X, OP.max)
        bp1 = small.tile([128, 16], F32, tag="bp1")
        nc.vector.tensor_scalar(bp1, bpiB, 1.0, None, OP.add)
        nc.vector.tensor_tensor(bp1, bp1, bad, OP.mult)
        nc.vector.tensor_tensor(bpiB, bpiB, bp1, OP.subtract)    # dups -> -1

        # --- forced override + per-prior best ---
        nc.vector.tensor_tensor(bx0, b3(cst["linf"]), t3(bpiB), OP.is_equal)  # fs
        nc.vector.scalar_tensor_tensor(binter, bx0, 2.0, binter, OP.mult, OP.add)  # iou2
        bff = work.tile([128, NP, 8], BF16, tag="bff")
        nc.vector.tensor_tensor(bff, binter[:, :, 0:8], binter[:, :, 8:16],
                                OP.max)
        nc.vector.tensor_tensor(bff[:, :, 0:4], bff[:, :, 0:4], bff[:, :, 4:8],
                                OP.max)
        bto = small.tile([128, NP], BF16, tag="bto")
        nc.vector.tensor_reduce(bto, bff[:, :, 0:4], AX.X, OP.max)
        # --- mask in [p, t, f] layout; payload gather via PE over maskT ---
        bmaskT = work.tile([128, 16, NP], BF16, tag="bmaskT")
        nc.vector.tensor_tensor(
            bmaskT, bur[:].rearrange("p f t -> p t f"),
            bto[:, None, :].to_broadcast([128, 16, NP]), OP.is_ge)
        smr = self.smask[r]
        nc.sync.dma_start(out=smr.rearrange("t p f -> p t f"), in_=bmaskT)
        mview = smr.rearrange("t p f -> t (p f)")
        mct = self.smct[r]
        NMM = PADP // 512                    # 49
        mi = 0
        while mi < NMM:
            gn = min(3, NMM - mi)
            w = gn * 512
            off0 = mi * 512
            mtk = small.tile([16, 1536], BF16, tag="mtk")
            nc.sync.dma_start(out=mtk[:, :w], in_=mview[:, off0 : off0 + w])
            mc_ps = self.psum_mc.tile([96, 512], F32, tag="mcps")
            for g in range(gn):
                nc.tensor.matmul(mc_ps[32 * g : 32 * g + 32, :], coordT,
                                 mtk[:, g * 512 : (g + 1) * 512],
                                 start=True, stop=True)
            stage = acc.tile([96, 512], F32, tag="stage")
            nc.scalar.copy(stage[: 32 * gn, :], mc_ps[: 32 * gn, :])
            nc.sync.dma_start(
                out=bass.AP(tensor=mct.tensor, offset=mct.offset + off0,
                            ap=[[512, gn], [PADP, 32], [1, 512]]),
                in_=stage[: 32 * gn, :])
            mi += gn
        mcrowf = work.tile([128, 5, NP], F32, tag="mcrowf")
        nc.sync.dma_start(
            out=mcrowf, in_=mct[:5].rearrange("ch (p f) -> p ch f", f=NP))
        nc.scalar.copy(self.mcres[r], mcrowf)

        # --- pos / num_pos ---
        pos = self.posr[r]
        nc.vector.scalar_tensor_tensor(pos, bto, 0.5, cst["validm"],
                                       OP.is_ge, OP.mult)
        nc.vector.tensor_reduce(self.npmat[:, r : r + 1], pos, AX.X, OP.add)

        # --- localization loss (moved into phase A; Ln table interleave
        #     already happens due to scheduler mixing, so no extra loads) ---
        lt = work.tile([128, NP, 4], F32, tag="blt")
        nc.vector.memset(lt[96:128, TAILF:, :], 0.0)
        nc.sync.dma_start(
            out=lt[:FULLP, :, :],
            in_=loc[r, : FULLP * NP, :].rearrange("(p f) c -> p f c", f=NP))
        nc.sync.dma_start(
            out=lt[FULLP : FULLP + 1, :TAILF, :],
            in_=loc[r, FULLP * NP : P, :].rearrange("(p f) c -> p f c", p=1))
        d4 = work.tile([128, NP, 4], BF16, tag="bd4")
        t196c = small.tile([128, NP], F32, tag="t196c")
        nc.vector.scalar_tensor_tensor(t196c, self.mcres[r][:, 0, :], 0.5,
                                       cst["pcx"], OP.mult, OP.subtract)
        nc.vector.tensor_tensor(t196c, t196c, cst["rw10"], OP.mult)
        nc.vector.tensor_tensor(d4[:, :, 0], lt[:, :, 0], t196c, OP.subtract)
        nc.vector.scalar_tensor_tensor(t196c, self.mcres[r][:, 2, :], 0.5,
                                       cst["pcy"], OP.mult, OP.subtract)
        nc.vector.tensor_tensor(t196c, t196c, cst["rh10"], OP.mult)
        nc.vector.tensor_tensor(d4[:, :, 1], lt[:, :, 1], t196c, OP.subtract)
        lnv = small.tile([128, NP], F32, tag="lnv")
        nc.scalar.activation(lnv, self.mcres[r][:, 1, :], AF.Ln)
        nc.vector.scalar_tensor_tensor(t196c, lnv, 5.0, cst["lnpw5"],
                                       OP.mult, OP.subtract)
        nc.vector.tensor_tensor(d4[:, :, 2], lt[:, :, 2], t196c, OP.subtract)
        nc.scalar.activation(lnv, self.mcres[r][:, 3, :], AF.Ln)
        nc.vector.scalar_tensor_tensor(t196c, lnv, 5.0, cst["lnph5"],
                                       OP.mult, OP.subtract)
        nc.vector.tensor_tensor(d4[:, :, 3], lt[:, :, 3], t196c, OP.subtract)
        nd = work.tile([128, NP, 4], BF16, tag="bnd")
        nc.vector.tensor_scalar(nd, d4, -1.0, None, OP.mult)
        nc.vector.tensor_tensor(d4, d4, nd, OP.max)              # |d|
        nc.vector.tensor_scalar(nd, d4, 1.0, None, OP.min)       # a
        nc.vector.scalar_tensor_tensor(d4, nd, -0.5, d4, OP.mult, OP.add)
        nc.vector.tensor_tensor(d4, d4, nd, OP.mult)             # smooth l1
        sl = small.tile([128, NP], F32, tag="sl")
        nc.vector.tensor_reduce(sl, d4, AX.X, OP.add)
        llrow = small.tile([128, 1], F32, tag="llrow")
        t196b = small.tile([128, NP], F32, tag="t196b")
        nc.vector.scalar_tensor_tensor(t196b, sl, 1.0, pos,
                                       OP.mult, OP.mult, accum_out=llrow)
        nc.vector.tensor_tensor(self.llacc, self.llacc, llrow, OP.add)

        # --- ct2 sentinel: labg+1 where pos else 200 ---
        posb = small.tile([128, NP], BF16, tag="posb")
        nc.vector.tensor_copy(posb, pos)
        ct2b = small.tile([128, NP], BF16, tag="ct2b")
        nc.vector.tensor_scalar(ct2b, self.mcres[r][:, 4, :], -199.0, None,
                                OP.add)
        nc.vector.tensor_tensor(ct2b, ct2b, posb, OP.mult)
        nc.vector.tensor_scalar(ct2b, ct2b, 200.0, None, OP.add)

        # --- conf chunks: exp / serow / c0 / conf_gt ---
        for ch in range(NCH):
            f0 = ch * GC
            i = (r * NCH + ch) % 2
            cf, et, mk = self.cf[i], self.et[i], self.mk[i]
            lastf = max(0, min(GC, TAILF - f0))
            nc.sync.dma_start(
                out=cf[:FULLP],
                in_=conf[r, : FULLP * NP, :]
                .rearrange("(p f) c -> p f c", f=NP)[:, f0 : f0 + GC, :])
            if lastf > 0:
                nc.sync.dma_start(
                    out=cf[FULLP : FULLP + 1, :lastf, :],
                    in_=conf[r, FULLP * NP + f0 : FULLP * NP + f0 + lastf, :]
                    .rearrange("(p f) c -> p f c", p=1))
            nc.scalar.activation(et, cf, AF.Exp)
            sfold = small.tile([128, GC, 40], BF16, tag="sfold")
            nc.vector.tensor_tensor(sfold, et[:, :, 1:41], et[:, :, 41:81],
                                    OP.add)
            nc.vector.tensor_tensor(sfold[:, :, 0:1], sfold[:, :, 0:1],
                                    et[:, :, 0:1], OP.add)
            nc.vector.tensor_tensor(sfold[:, :, 0:20], sfold[:, :, 0:20],
                                    sfold[:, :, 20:40], OP.add)
            nc.vector.tensor_reduce(self.serow[r][:, f0 : f0 + GC],
                                    sfold[:, :, 0:20], AX.X, OP.add)
            nc.scalar.copy(self.c0row[r][:, f0 : f0 + GC], cf[:, :, 0])
            nc.vector.tensor_tensor(
                mk, self.cst["io81b"][:, None, :].to_broadcast([128, GC, C]),
                ct2b[:, f0 : f0 + GC, None].to_broadcast([128, GC, C]),
                OP.is_equal)
            cfb = self.cfb[i]
            nc.scalar.copy(cfb, cf)
            bcp = small.tile([128, 1], F32, tag="bcp")
            nc.vector.scalar_tensor_tensor(mk, mk, 1.0, cfb, OP.mult,
                                           OP.mult, accum_out=bcp)
            nc.vector.tensor_tensor(self.bcacc, self.bcacc, bcp, OP.add)

    # ------------------------------------------------------------------
    def rowB(self, r, loc, scratch):
        nc = self.nc
        cst = self.cst
        work = self.work
        small = self.small

        # --- lse, blrow, loss_c ---
        lse = acc.tile([128, NP], F32, tag="lse")
        nc.scalar.activation(lse, self.serow[r], AF.Ln)
        t196 = acc.tile([128, NP], F32, tag="t196")
        blrow = small.tile([128, 1], F32, tag="blrow")
        nc.vector.scalar_tensor_tensor(t196, lse, 1.0, self.posr[r],
                                       OP.mult, OP.mult, accum_out=blrow)
        nc.vector.tensor_tensor(self.blacc, self.blacc, blrow, OP.add)
        lcv = acc.tile([128, NP], F32, tag="lcv")
        nc.vector.tensor_tensor(lcv, lse, self.c0row[r], OP.subtract)
        np1 = acc.tile([128, NP], F32, tag="np1")
        nc.vector.tensor_scalar(np1, self.posr[r], -1.0, 1.0, OP.mult, OP.add)
        nc.vector.tensor_tensor(lcv, lcv, np1, OP.mult)
        # pads -> -1 (never mined): lc = (lc+1)*valid - 1
        nc.vector.tensor_scalar(lcv, lcv, 1.0, None, OP.add)
        nc.vector.tensor_tensor(lcv, lcv, cst["validm"], OP.mult)
        lcb = small.tile([128, NP], BF16, tag="lcb")
        nc.vector.tensor_scalar(lcb, lcv, -1.0, None, OP.add)
        nc.sync.dma_start(
            out=scratch[r].rearrange("(p f) -> p f", f=NP), in_=lcb)

    # ------------------------------------------------------------------
    def mining(self, scratch, out):
        nc = self.nc
        cst = self.cst
        small = self.small
        acc = self.acc
        psum = self.psum
        G128, sel8, selone = cst["G128"], cst["sel8"], cst["selone"]
        ones_128x1 = cst["ones_128x1"]

        # per-row num_pos totals: [8,1] = npmat^T @ ones
        np_ps = psum.tile([8, 1], F32, tag="ps")
        nc.tensor.matmul(np_ps, self.npmat, ones_128x1, start=True, stop=True)
        npv = small.tile([8, 1], F32, tag="npv")
        nc.scalar.copy(npv, np_ps)
        e_ps = psum.tile([1, 1], F32, tag="ps")
        nc.tensor.matmul(e_ps, npv, ones_128x1[:8, :], start=True, stop=True)
        kv = small.tile([8, 1], F32, tag="kv")
        nc.vector.tensor_scalar(kv, npv, 3.0, float(P - 1), OP.mult, OP.min)
        nc.sync.dma_start(out=self.sk8.rearrange("o e -> (o e) ()"), in_=kv)
        kb = small.tile([128, 8], F32, tag="kb")
        nc.sync.dma_start(
            out=kb,
            in_=bass.AP(tensor=self.sk8.tensor, offset=0,
                        ap=[[0, 128], [1, 8]]))
        k128 = small.tile([128, 1], F32, tag="k128")
        ks = small.tile([128, 8], F32, tag="ks")
        nc.vector.scalar_tensor_tensor(ks, kb, 1.0, sel8, OP.mult, OP.mult,
                                       accum_out=k128)

        # loss_c packed [128, 1568] bf16
        lcp = acc.tile([128, MCH], BF16, tag="lcp")
        nc.sync.dma_start(
            out=lcp,
            in_=bass.AP(tensor=scratch.tensor, offset=0,
                        ap=[[MCH, 128], [1, MCH]]))

        lo = small.tile([128, 1], F32, tag="lo")
        nc.vector.memset(lo, 0.0)
        hi = small.tile([128, 1], F32, tag="hi")
        nc.vector.memset(hi, 12.0)
        mid = small.tile([128, 1], F32, tag="mid")
        msk = acc.tile([128, MCH], BF16, tag="msk")
        for it in range(N_ITERS):
            nc.vector.tensor_tensor(mid, lo, hi, OP.add)
            nc.scalar.mul(mid, mid, 0.5)
            pc = small.tile([128, 1], F32, tag="pc")
            nc.vector.tensor_scalar(msk, lcp, mid[:, 0:1], None, OP.is_gt,
                                    OP.add, accum_out=pc)
            c_ps = psum.tile([128, 1], F32, tag="ps")
            nc.tensor.matmul(c_ps, G128, pc, start=True, stop=True)
            sel = small.tile([128, 1], U8, tag="sel")
            nc.vector.tensor_scalar(sel, c_ps, k128[:, 0:1], None, OP.is_ge)
            nc.vector.copy_predicated(lo, sel, mid)
            sel2 = small.tile([128, 1], U8, tag="sel2")
            nc.vector.tensor_scalar(sel2, c_ps, k128[:, 0:1], None, OP.is_lt)
            nc.vector.copy_predicated(hi, sel2, mid)

        # final masked sum + count at threshold lo
        st2 = small.tile([128, 2], F32, tag="st2")
        nc.vector.scalar_tensor_tensor(msk, lcp, lo[:, 0:1], lcp, OP.is_gt,
                                       OP.mult, accum_out=st2[:, 0:1])
        nc.vector.tensor_scalar(msk, lcp, lo[:, 0:1], None, OP.is_gt,
                                OP.add, accum_out=st2[:, 1:2])
        g2_ps = psum.tile([128, 2], F32, tag="ps")
        nc.tensor.matmul(g2_ps, G128, st2, start=True, stop=True)
        gt2 = small.tile([128, 2], F32, tag="gt2")
        nc.scalar.copy(gt2, g2_ps)
        sn = small.tile([128, 1], F32, tag="sn")
        nc.vector.tensor_tensor(sn, gt2[:, 1:2], k128, OP.subtract)
        nc.vector.tensor_tensor(sn, sn, lo, OP.mult)
        nc.vector.tensor_tensor(sn, gt2[:, 0:1], sn, OP.subtract)
        d_ps = psum.tile([1, 1], F32, tag="ps")
        nc.tensor.matmul(d_ps, sn, selone, start=True, stop=True)

        # final scalars A..E
        a_ps = psum.tile([1, 1], F32, tag="ps")
        nc.tensor.matmul(a_ps, self.llacc, ones_128x1, start=True, stop=True)
        b_ps = psum.tile([1, 1], F32, tag="ps")
        nc.tensor.matmul(b_ps, self.blacc, ones_128x1, start=True, stop=True)
        c2_ps = psum.tile([1, 1], F32, tag="ps")
        nc.tensor.matmul(c2_ps, self.bcacc, ones_128x1, start=True, stop=True)
        outsb = small.tile([1, 8], F32, tag="outsb")
        nc.vector.memset(outsb, 0.0)
        nc.scalar.copy(outsb[:, 0:1], a_ps)
        nc.scalar.copy(outsb[:, 1:2], b_ps)
        nc.scalar.copy(outsb[:, 2:3], c2_ps)
        nc.scalar.copy(outsb[:, 3:4], d_ps)
        nc.scalar.copy(outsb[:, 4:5], e_ps)
        nc.sync.dma_start(out=out, in_=outsb)


_CACHED = {}


def kernel(loc_data, conf_data, priors, targets):
    if "nc" not in _CACHED:
        _CACHED["nc"] = build_program()
    nc = _CACHED["nc"]
    in_maps = []
    for c in range(NCORES):
        sl = slice(c * R, (c + 1) * R)
        in_maps.append({
            "loc": np.ascontiguousarray(loc_data[sl]),
            "conf": np.ascontiguousarray(conf_data[sl]),
            "priors": np.ascontiguousarray(priors),
            "targets": np.ascontiguousarray(targets[sl]),
        })
    res = bass_utils.run_bass_kernel_spmd(nc, in_maps, core_ids=list(range(NCORES)))
    _CACHED["last_results"] = res
    A = Bs = Cs = D = E = 0.0
    for c in range(NCORES):
        o = res.results[c]["out"].reshape(-1)
        A += float(o[0]); Bs += float(o[1]); Cs += float(o[2])
        D += float(o[3]); E += float(o[4])
    N = max(E, 1.0)
    return np.array([A / N, (Bs - Cs + D) / N], dtype=np.float32)
